# revision 17
# baseline (speedup 1.0000x reference)
"""DeepseekV2 MLA attention (prefill, causal) on 8 trn2 NeuronCores — v5.

Tensor-parallel over heads (2 heads/core), sequence-parallel shared
projections, like the fp32 baseline — but:

* bf16 everywhere on-chip (fp32 PSUM accumulation); weights converted
  host-side, x transposed host-side.  Halves DMA, SBUF and collective
  bytes; same PE throughput (fp32r was already full rate).
* Collectives are AllGather / ReduceScatter only (AllToAll hits an
  intermittent ~8ms/MB slow mode on this platform).  The AllGather is
  fed through a same-queue gpsimd copy (collectives waiting directly on
  cross-queue HWDGE semaphores triggered the same pathology).
* The ReduceScatter runs in bf16, halving the dominant 16.8 MB output
  exchange of the baseline.
* Causal masking by binary bf16 mask-multiply on the exp() output
  (|scores| < 8 for this operator family, exp cannot overflow).
* Softmax on transposed scores [k | q]: no max subtraction, denominators
  from an all-ones matmul which also pre-broadcasts over partitions.
"""
import sys

sys.path.insert(0, "/opt/trn_rl_repo")

import numpy as np

import concourse.bass as bass
from concourse import bacc
import concourse.mybir as mybir
import concourse.tile as tile
from concourse.bass_utils import run_bass_kernel_spmd

F32 = mybir.dt.float32
BF = mybir.dt.bfloat16
AF = mybir.ActivationFunctionType

B, S, E, H = 1, 2048, 2048, 16
DN, DR, DV, R, QLR = 128, 64, 128, 512, 1536
EPS = 1e-6
NCORES = 8
TOK = S // NCORES         # 256 tokens per core
HPC = H // NCORES         # 2 heads per core
SM_SCALE = (DN + DR) ** -0.5
ROPE_BASE = 10000.0

EC = E // 128             # 16 contraction chunks over E
QRC = QLR // 128          # 12 chunks of q_a features
CRC = R // 128            # 4 chunks of ckv features
NQC = S // 512            # 4 query column chunks
NKT = S // 128            # 16 key tiles
AGR = QLR + R + DR        # 2112 rows in the AllGather payload


def _rope_tables():
    inv_freq = 1.0 / (ROPE_BASE ** (np.arange(0, DR, 2, dtype=np.float64) / DR))
    ang = np.arange(S, dtype=np.float64)[:, None] * inv_freq[None, :]
    cos = np.concatenate([np.cos(ang), np.cos(ang)], -1).astype(np.float32)  # [S,DR]
    sin = np.concatenate([np.sin(ang), np.sin(ang)], -1).astype(np.float32)
    return cos.T.copy(), sin.T.copy()  # [DR, S] feature-major


def _consts():
    p = np.zeros((64, 64), dtype=np.float32)
    for j in range(32):
        p[j + 32, j] = -1.0
    for j in range(32, 64):
        p[j - 32, j] = 1.0
    prot = np.zeros((128, 128), dtype=np.float32)
    prot[:64, :64] = p
    prot[64:, 64:] = p
    ii = np.arange(128)[:, None]
    jj = np.arange(512)[None, :]
    masks = np.stack(
        [np.where(jj - ii - 128 * m >= 0, 1.0, 0.0).astype(np.float32)
         for m in range(4)])
    cosT, sinT = _rope_tables()
    cos2 = np.concatenate([cosT, cosT], 0)  # [128, S] two stacked heads
    sin2 = np.concatenate([sinT, sinT], 0)
    bf = mybir.dt.np(BF)
    return prot.astype(bf), masks.astype(bf), cos2, sin2


def _build(skip_collectives=False):
    nc = bacc.Bacc(None, num_devices=NCORES)

    xT_sl = nc.dram_tensor("xT_sl", [E, TOK], BF, kind="ExternalInput")
    w_qa = nc.dram_tensor("w_qa", [E, QLR], BF, kind="ExternalInput")
    w_kva = nc.dram_tensor("w_kva", [E, R + DR], BF, kind="ExternalInput")
    w_qb_sl = nc.dram_tensor("w_qb_sl", [QLR, HPC * (DN + DR)], BF,
                             kind="ExternalInput")
    w_uk_sl = nc.dram_tensor("w_uk_sl", [R, HPC * DN], BF, kind="ExternalInput")
    w_uv_sl = nc.dram_tensor("w_uv_sl", [R, HPC * DV], BF, kind="ExternalInput")
    w_o_sl = nc.dram_tensor("w_o_sl", [HPC * DV, E], BF, kind="ExternalInput")
    lnw_q = nc.dram_tensor("lnw_q", [QLR, 1], F32, kind="ExternalInput")
    lnw_kv = nc.dram_tensor("lnw_kv", [R, 1], F32, kind="ExternalInput")
    cos_sl = nc.dram_tensor("cos_sl", [64, TOK], F32, kind="ExternalInput")
    sin_sl = nc.dram_tensor("sin_sl", [64, TOK], F32, kind="ExternalInput")
    ones_in = nc.dram_tensor("ones_in", [128, 128], BF, kind="ExternalInput")
    y_sl = nc.dram_tensor("y_sl", [TOK, E], F32, kind="ExternalOutput")

    prot_np, masks_np, cos2_np, sin2_np = _consts()
    prot_t = nc.inline_tensor(prot_np, name="prot_c")
    masks_t = nc.inline_tensor(masks_np, name="masks_c")
    cos2_t = nc.inline_tensor(cos2_np, name="cos2_c")
    sin2_t = nc.inline_tensor(sin2_np, name="sin2_c")

    agkv_in = nc.dram_tensor("agkv_in", [R + DR, TOK], BF)
    agkv_mid = nc.dram_tensor("agkv_mid", [R + DR, TOK], BF)
    agkv_out = nc.dram_tensor("agkv_out", [NCORES * (R + DR), TOK], BF,
                              addr_space="Shared")
    agqa_in = nc.dram_tensor("agqa_in", [QLR, TOK], BF)
    agqa_mid = nc.dram_tensor("agqa_mid", [QLR, TOK], BF)
    agqa_out = nc.dram_tensor("agqa_out", [NCORES * QLR, TOK], BF,
                              addr_space="Shared")
    rs_in = nc.dram_tensor("rs_in", [S, E], BF)
    rs_mid = nc.dram_tensor("rs_mid", [S, E], BF)
    rs_out = nc.dram_tensor("rs_out", [TOK, E], BF)

    with tile.TileContext(nc) as tc:
        with tc.tile_pool(name="consts", bufs=1) as cp:
            ones_sb = cp.tile([128, 128], BF)
            nc.sync.dma_start(out=ones_sb, in_=ones_in[:, :])
            prot_sb = cp.tile([128, 128], BF)
            nc.sync.dma_start(out=prot_sb, in_=prot_t[:, :])
            eps_sb = cp.tile([128, 1], F32)
            nc.vector.memset(eps_sb[:], EPS)
            lnwq_sb = cp.tile([128, QRC], F32)
            nc.sync.dma_start(
                out=lnwq_sb, in_=lnw_q.rearrange("(rc p) one -> p rc one", p=128))
            lnwkv_sb = cp.tile([128, CRC], F32)
            nc.sync.dma_start(
                out=lnwkv_sb, in_=lnw_kv.rearrange("(rc p) one -> p rc one", p=128))
            cos_sb = cp.tile([64, TOK], F32)
            nc.sync.dma_start(out=cos_sb, in_=cos_sl[:, :])
            sin_sb = cp.tile([64, TOK], F32)
            nc.sync.dma_start(out=sin_sb, in_=sin_sl[:, :])

            # ---------------- stage A: q_a / ckv / k_pe on own tokens --------
            with tc.tile_pool(name="pa", bufs=2) as pa, \
                 tc.tile_pool(name="psA", bufs=2, space="PSUM") as psA:
                xT = pa.tile([128, EC, TOK], BF, tag="xT", bufs=1)
                nc.sync.dma_start(
                    out=xT, in_=xT_sl.rearrange("(kc p) s -> p kc s", p=128))
                wkva_sb = pa.tile([128, EC, R + DR], BF, tag="wkva", bufs=1)
                nc.sync.dma_start(
                    out=wkva_sb, in_=w_kva.rearrange("(kc p) m -> p kc m", p=128))
                wqa_sb = pa.tile([128, EC, QLR], BF, tag="wqa", bufs=1)
                nc.sync.dma_start(
                    out=wqa_sb, in_=w_qa.rearrange("(kc p) m -> p kc m", p=128))

                qa_n = pa.tile([128, QRC, TOK], BF, tag="qa_n", bufs=1)
                ckv_n = pa.tile([128, CRC, TOK], BF, tag="ckv_n", bufs=1)
                kpe_out = pa.tile([64, TOK], BF, tag="kpe_out", bufs=1)

                def feat_major_block(w_sb, col_off, nchunks, lnw_sb, nfeat,
                                     out_tile):
                    raw = pa.tile([128, nchunks, TOK], BF, tag=f"raw{nfeat}",
                                  bufs=1)
                    ssq = psA.tile([128, TOK], F32, tag="ssq", bufs=1)
                    for rc in range(nchunks):
                        pq = psA.tile([128, TOK], F32, tag="pq", bufs=2)
                        for kc in range(EC):
                            nc.tensor.matmul(
                                pq[:],
                                w_sb[:, kc, col_off + rc * 128:
                                     col_off + (rc + 1) * 128],
                                xT[:, kc, :],
                                start=(kc == 0), stop=(kc == EC - 1))
                        if rc % 2 == 0:
                            nc.vector.tensor_copy(raw[:, rc, :], pq[:])
                        else:
                            nc.scalar.copy(raw[:, rc, :], pq[:])
                        sq = pa.tile([128, TOK], BF, tag="sq", bufs=2)
                        nc.scalar.activation(out=sq, in_=pq[:], func=AF.Square)
                        nc.tensor.matmul(ssq[:], ones_sb[:], sq[:],
                                         start=(rc == 0), stop=(rc == nchunks - 1))
                    rstd = pa.tile([128, TOK], F32, tag=f"rstd{nfeat}", bufs=1)
                    nc.scalar.activation(out=rstd, in_=ssq[:], func=AF.Sqrt,
                                         scale=1.0 / nfeat, bias=eps_sb[:])
                    nc.vector.reciprocal(rstd[:], rstd[:])
                    for rc in range(nchunks):
                        nc.vector.scalar_tensor_tensor(
                            out=out_tile[:, rc, :], in0=raw[:, rc, :],
                            scalar=lnw_sb[:, rc:rc + 1], in1=rstd[:],
                            op0=mybir.AluOpType.mult, op1=mybir.AluOpType.mult)

                feat_major_block(wkva_sb, 0, CRC, lnwkv_sb, R, ckv_n)
                nc.sync.dma_start(
                    out=agkv_in[0:R, :].rearrange("(rc p) s -> p rc s", p=128),
                    in_=ckv_n[:, :, :])

                # k_pe + rope
                with tc.tile_pool(name="psPE", bufs=1, space="PSUM") as psPE:
                    ppe = psPE.tile([64, TOK], F32, tag="ppe", bufs=1)
                    for kc in range(EC):
                        nc.tensor.matmul(ppe[:], wkva_sb[:, kc, R:R + DR],
                                         xT[:, kc, :],
                                         start=(kc == 0), stop=(kc == EC - 1))
                    kpe_raw = pa.tile([64, TOK], BF, tag="kpe_raw", bufs=1)
                    nc.scalar.copy(kpe_raw[:], ppe[:])
                    prot_ps = psPE.tile([64, TOK], F32, tag="prot_ps", bufs=1)
                    nc.tensor.matmul(prot_ps[:], prot_sb[0:64, 0:64], kpe_raw[:],
                                     start=True, stop=True)
                    t1 = pa.tile([64, TOK], F32, tag="t1", bufs=1)
                    nc.vector.tensor_mul(t1[:], kpe_raw[:], cos_sb[:])
                    t2 = pa.tile([64, TOK], F32, tag="t2", bufs=1)
                    nc.vector.tensor_mul(t2[:], prot_ps[:], sin_sb[:])
                    nc.vector.tensor_add(kpe_out[:], t1[:], t2[:])
                nc.sync.dma_start(out=agkv_in[R:R + DR, :], in_=kpe_out[:])
                # kv AllGather ships while q_a is still computing.  The
                # same-queue feeder copy matters: collectives waiting directly
                # on cross-queue HWDGE semaphores hit an intermittent slow mode.
                if skip_collectives:
                    for c in range(NCORES):
                        nc.gpsimd.dma_start(
                            out=agkv_out[c * (R + DR):(c + 1) * (R + DR), :],
                            in_=agkv_in[:, :])
                else:
                    nc.gpsimd.dma_start(out=agkv_mid[:, :], in_=agkv_in[:, :])
                    nc.gpsimd.collective_compute(
                        "AllGather", mybir.AluOpType.bypass,
                        replica_groups=[list(range(NCORES))],
                        ins=[agkv_mid[:, :].opt()], outs=[agkv_out[:, :].opt()])

                feat_major_block(wqa_sb, 0, QRC, lnwq_sb, QLR, qa_n)
                nc.sync.dma_start(
                    out=agqa_in[:, :].rearrange("(rc p) s -> p rc s", p=128),
                    in_=qa_n[:, :, :])

            if skip_collectives:
                for c in range(NCORES):
                    nc.gpsimd.dma_start(
                        out=agqa_out[c * QLR:(c + 1) * QLR, :], in_=agqa_in[:, :])
            else:
                nc.gpsimd.dma_start(out=agqa_mid[:, :], in_=agqa_in[:, :])
                nc.gpsimd.collective_compute(
                    "AllGather", mybir.AluOpType.bypass,
                    replica_groups=[list(range(NCORES))],
                    ins=[agqa_mid[:, :].opt()], outs=[agqa_out[:, :].opt()])

            kvv = agkv_out.rearrange("(c r) s -> r c s", c=NCORES)
            qav = agqa_out.rearrange("(c r) s -> r c s", c=NCORES)

            # -------- stage B: q (own heads, all tokens), knT, v --------
            with tc.tile_pool(name="ab", bufs=1) as ab:
                psB_cm = tc.tile_pool(name="psB", bufs=2, space="PSUM")
                psB = psB_cm.__enter__()
                cos2_sb = ab.tile([128, S], F32, tag="cos2")
                nc.scalar.dma_start(out=cos2_sb, in_=cos2_t[:, :])
                sin2_sb = ab.tile([128, S], F32, tag="sin2")
                nc.scalar.dma_start(out=sin2_sb, in_=sin2_t[:, :])
                mask_sb = ab.tile([128, 4, 512], BF, tag="mask")
                nc.scalar.dma_start(out=mask_sb,
                                    in_=masks_t.rearrange("m p f -> p m f"))
                wqb_sb = ab.tile([128, QRC, HPC * (DN + DR)], BF, tag="wqb")
                nc.scalar.dma_start(
                    out=wqb_sb, in_=w_qb_sl.rearrange("(kc p) m -> p kc m", p=128))
                wuk_sb = ab.tile([128, CRC, HPC * DN], BF, tag="wuk")
                nc.scalar.dma_start(
                    out=wuk_sb, in_=w_uk_sl.rearrange("(rc p) m -> p rc m", p=128))
                wuv_sb = ab.tile([128, CRC, HPC * DV], BF, tag="wuv")
                nc.scalar.dma_start(
                    out=wuv_sb, in_=w_uv_sl.rearrange("(rc p) m -> p rc m", p=128))
                wo_sb = ab.tile([128, HPC, E], BF, tag="wo")
                nc.scalar.dma_start(
                    out=wo_sb, in_=w_o_sl.rearrange("(hc p) e -> p hc e", p=128))

                # gathered kv activations (available during the q_a gather)
                ckvT = ab.tile([128, CRC, S], BF, tag="ckvT")
                for rc in range(CRC):
                    nc.sync.dma_start(
                        out=ckvT[:, rc, :],
                        in_=kvv[rc * 128:(rc + 1) * 128, :, :])
                kpeT = ab.tile([64, S], BF, tag="kpeT")
                nc.sync.dma_start(out=kpeT, in_=kvv[R:R + DR, :, :])

                knT = [ab.tile([128, S], BF, tag=f"knT{h}", name=f"knT{h}")
                       for h in range(HPC)]
                for h in range(HPC):
                    for nq in range(NQC):
                        pk = psB.tile([128, 512], F32, tag="pk", bufs=2)
                        for rc in range(CRC):
                            nc.tensor.matmul(
                                pk[:], wuk_sb[:, rc, h * DN:(h + 1) * DN],
                                ckvT[:, rc, nq * 512:(nq + 1) * 512],
                                start=(rc == 0), stop=(rc == CRC - 1))
                        if h == 0:
                            nc.vector.tensor_copy(
                                knT[h][:, nq * 512:(nq + 1) * 512], pk[:])
                        else:
                            nc.scalar.copy(
                                knT[h][:, nq * 512:(nq + 1) * 512], pk[:])

                v_sb = ab.tile([128, NKT, HPC * DV], BF, tag="v_sb")
                for kt in range(NKT):
                    pv = psB.tile([128, HPC * DV], F32, tag="pv", bufs=2)
                    for rc in range(CRC):
                        nc.tensor.matmul(
                            pv[:], ckvT[:, rc, kt * 128:(kt + 1) * 128],
                            wuv_sb[:, rc, :], start=(rc == 0),
                            stop=(rc == CRC - 1))
                    if kt % 2 == 0:
                        nc.vector.tensor_copy(v_sb[:, kt, :], pv[:])
                    else:
                        nc.scalar.copy(v_sb[:, kt, :], pv[:])

                qaT = ab.tile([128, QRC, S], BF, tag="qaT")
                for rc in range(QRC):
                    nc.sync.dma_start(out=qaT[:, rc, :],
                                      in_=qav[rc * 128:(rc + 1) * 128, :, :])
                # q projection + rope (3 chunks of 128 out-features per 512 q)
                qnT = [ab.tile([128, S], BF, tag=f"qnT{h}", name=f"qnT{h}")
                       for h in range(HPC)]
                qpeT = ab.tile([128, S], BF, tag="qpeT")
                for qc in range(NQC):
                    cs = slice(qc * 512, (qc + 1) * 512)
                    for mc in range(3):
                        pq2 = psB.tile([128, 512], F32, tag="pq2", bufs=2)
                        for kc in range(QRC):
                            nc.tensor.matmul(
                                pq2[:], wqb_sb[:, kc, mc * 128:(mc + 1) * 128],
                                qaT[:, kc, cs],
                                start=(kc == 0), stop=(kc == QRC - 1))
                        if mc < 2:
                            if mc == 0:
                                nc.vector.tensor_copy(qnT[mc][:, cs], pq2[:])
                            else:
                                nc.scalar.copy(qnT[mc][:, cs], pq2[:])
                        else:
                            qpe_raw = ab.tile([128, 512], BF, tag="qpe_raw",
                                              bufs=2)
                            nc.scalar.copy(qpe_raw[:], pq2[:])
                            rot_ps = psB.tile([128, 512], F32, tag="rot_ps",
                                              bufs=2)
                            nc.tensor.matmul(rot_ps[:], prot_sb[:], qpe_raw[:],
                                             start=True, stop=True)
                            tq1 = ab.tile([128, 512], F32, tag="tq1", bufs=2)
                            nc.gpsimd.tensor_mul(tq1[:], qpe_raw[:],
                                                 cos2_sb[:, cs])
                            tq2 = ab.tile([128, 512], F32, tag="tq2", bufs=2)
                            nc.vector.tensor_mul(tq2[:], rot_ps[:],
                                                 sin2_sb[:, cs])
                            nc.vector.tensor_add(qpeT[:, cs], tq1[:], tq2[:])
                qpe_h1 = ab.tile([64, S], BF, tag="qpe_h1")
                nc.sync.dma_start(out=qpe_h1, in_=qpeT[64:128, :])


                psB_cm.__exit__(None, None, None)
                # ------------- stage C: attention + partial y -------------
                with tc.tile_pool(name="psC", bufs=1, space="PSUM") as psC:
                    for qc in range(NQC):
                        cs = slice(qc * 512, (qc + 1) * 512)
                        nkt = 4 * qc + 4
                        ofins = []
                        for h in range(HPC):
                            po = psC.tile([128, 512], F32, tag="po", bufs=2)
                            pdn = psC.tile([128, 512], F32, tag="pdn", bufs=2)
                            for kt in range(nkt):
                                ks = slice(kt * 128, (kt + 1) * 128)
                                ps = psC.tile([128, 512], F32, tag="ps", bufs=2)
                                nc.tensor.matmul(ps[:], knT[h][:, ks],
                                                 qnT[h][:, cs],
                                                 start=True, stop=False)
                                qpe_rhs = (qpeT[0:64, cs] if h == 0
                                           else qpe_h1[:, cs])
                                nc.tensor.matmul(ps[:], kpeT[:, ks], qpe_rhs,
                                                 start=False, stop=True)
                                et = ab.tile([128, 512], BF, tag="et", bufs=3)
                                nc.scalar.activation(out=et, in_=ps[:],
                                                     func=AF.Exp,
                                                     scale=SM_SCALE)
                                m = kt - 4 * qc
                                if m >= 0:
                                    nc.gpsimd.tensor_mul(et[:], et[:],
                                                         mask_sb[:, m, :])
                                nc.tensor.matmul(po[:],
                                                 v_sb[:, kt, h * DV:(h + 1) * DV],
                                                 et[:], start=(kt == 0),
                                                 stop=(kt == nkt - 1))
                                nc.tensor.matmul(pdn[:], ones_sb[:], et[:],
                                                 start=(kt == 0),
                                                 stop=(kt == nkt - 1))
                            rec = ab.tile([128, 512], F32, tag="rec", bufs=2)
                            nc.vector.reciprocal(rec[:], pdn[:])
                            ofin = ab.tile([128, 512], BF, tag=f"ofin{h}",
                                           name=f"ofin{h}", bufs=2)
                            nc.vector.tensor_mul(ofin[:], po[:], rec[:])
                            ofins.append(ofin)
                        # partial y for these 512 q rows -> rs_in (bf16)
                        for mc in range(4):
                            ms = slice(mc * 128, (mc + 1) * 128)
                            for nq in range(NQC):
                                py = psC.tile([128, 512], F32, tag="py", bufs=2)
                                for h in range(HPC):
                                    nc.tensor.matmul(
                                        py[:], ofins[h][:, ms],
                                        wo_sb[:, h, nq * 512:(nq + 1) * 512],
                                        start=(h == 0), stop=(h == HPC - 1))
                                y_sb = ab.tile([128, 512], BF, tag="y_sb",
                                               bufs=3)
                                if nq % 2 == 0:
                                    nc.vector.tensor_copy(y_sb[:], py[:])
                                else:
                                    nc.scalar.copy(y_sb[:], py[:])
                                nc.sync.dma_start(
                                    out=rs_in[qc * 512 + mc * 128:
                                              qc * 512 + (mc + 1) * 128,
                                              nq * 512:(nq + 1) * 512],
                                    in_=y_sb[:])
                        # reduce-scatter this q-chunk while attention continues.
                        # rs_out row block qc holds rows [qc*512 + 64*core ..
                        # +64) of the global sum; the host reassembles.
                        cl = slice(qc * 512, (qc + 1) * 512)
                        if skip_collectives:
                            nc.gpsimd.dma_start(
                                out=rs_out[qc * 64:(qc + 1) * 64, :],
                                in_=rs_in[qc * 512:qc * 512 + 64, :])
                        else:
                            nc.gpsimd.dma_start(out=rs_mid[cl, :],
                                                in_=rs_in[cl, :])
                            nc.gpsimd.collective_compute(
                                "ReduceScatter", mybir.AluOpType.add,
                                replica_groups=[list(range(NCORES))],
                                ins=[rs_mid[cl, :].opt()],
                                outs=[rs_out[qc * 64:(qc + 1) * 64, :].opt()])


                # bf16 -> f32 output
                yb = ab.tile([128, 2, E], BF, tag="yb")
                nc.sync.dma_start(
                    out=yb, in_=rs_out.rearrange("(c p) e -> p c e", p=128))
                yf = ab.tile([128, 2, E], F32, tag="yf")
                nc.vector.tensor_copy(yf[:], yb[:])
                nc.sync.dma_start(
                    out=y_sl.rearrange("(c p) e -> p c e", p=128), in_=yf[:])
    nc.finalize()
    return nc


_NC_CACHE = None


def _get_nc():
    global _NC_CACHE
    if _NC_CACHE is None:
        _NC_CACHE = _build()
    return _NC_CACHE


def _make_in_maps(x, w_q_a, q_a_ln_w, w_q_b, w_kv_a, kv_a_ln_w, w_kv_b, w_o):
    bf = mybir.dt.np(BF)
    x = np.asarray(x, dtype=np.float32)
    w_qa_b = np.ascontiguousarray(np.asarray(w_q_a, np.float32)).astype(bf)
    w_kva_b = np.ascontiguousarray(np.asarray(w_kv_a, np.float32)).astype(bf)
    wqb = np.asarray(w_q_b, np.float32).reshape(QLR, H, DN + DR)
    wkv = np.asarray(w_kv_b, np.float32).reshape(R, H, DN + DV)
    w_o = np.asarray(w_o, np.float32)

    cosT, sinT = _rope_tables()
    in_maps = []
    for c in range(NCORES):
        h0, h1 = HPC * c, HPC * c + 1
        sl = slice(c * TOK, (c + 1) * TOK)
        w_qb_sl = np.concatenate(
            [wqb[:, h0, :DN], wqb[:, h1, :DN], wqb[:, h0, DN:], wqb[:, h1, DN:]],
            axis=1)
        w_uk_sl = np.concatenate([wkv[:, h0, :DN], wkv[:, h1, :DN]], axis=1)
        w_uv_sl = np.concatenate([wkv[:, h0, DN:], wkv[:, h1, DN:]], axis=1)
        in_maps.append({
            "xT_sl": np.ascontiguousarray(x[0, sl, :].T).astype(bf),
            "w_qa": w_qa_b,
            "w_kva": w_kva_b,
            "w_qb_sl": np.ascontiguousarray(w_qb_sl).astype(bf),
            "w_uk_sl": np.ascontiguousarray(w_uk_sl).astype(bf),
            "w_uv_sl": np.ascontiguousarray(w_uv_sl).astype(bf),
            "w_o_sl": np.ascontiguousarray(
                w_o[h0 * DV:(h1 + 1) * DV, :]).astype(bf),
            "lnw_q": np.ascontiguousarray(
                np.asarray(q_a_ln_w, np.float32).reshape(QLR, 1)),
            "lnw_kv": np.ascontiguousarray(
                np.asarray(kv_a_ln_w, np.float32).reshape(R, 1)),
            "cos_sl": np.ascontiguousarray(cosT[:, sl]),
            "sin_sl": np.ascontiguousarray(sinT[:, sl]),
            "ones_in": np.ones((128, 128), np.float32).astype(bf),
        })
    return in_maps


def kernel(**inputs):
    in_maps = _make_in_maps(**inputs)
    nc = _get_nc()
    # The axon terminal occasionally reports NRT_EXEC_UNIT_UNRECOVERABLE on the
    # first load after a prior session died; a retry recovers it.
    last_exc = None
    for _ in range(3):
        try:
            res = run_bass_kernel_spmd(nc, in_maps, core_ids=list(range(NCORES)))
            break
        except Exception as e:  # noqa: BLE001
            last_exc = e
    else:
        raise last_exc
    # y_sl row block qc on core c = global rows [qc*512 + 64*c, qc*512+64*c+64)
    y = np.empty((S, E), np.float32)
    for c in range(NCORES):
        ysl = res.results[c]["y_sl"]
        for qc in range(4):
            y[qc * 512 + 64 * c:qc * 512 + 64 * c + 64] = \
                ysl[qc * 64:(qc + 1) * 64]
    return y.reshape(B, S, E).astype(np.float32)


if __name__ == "__main__":
    nc = _build()
    print("built ok")
